# revision 17
# baseline (speedup 1.0000x reference)
"""Trainium2 Bass kernel for nn_Block_33105607917736 (sparse_attention).

Reference block:
  x = x*m
  x = (x + local_attn(LN1(x))) * m      # sliding-window attn, WL=128, 2*WL keys
  x = (x + global_attn(LN2(x))) * m     # dilated attn, stride WG=64, seq len 128
  x = (x + LN3(x) @ ffn_w + ffn_b) * m

Sharding: 8 cores = (batch b, half h).  Two SPMD launches (no collectives):
  Launch 1: core (b,h) computes y1 = x + local_attn(LN1(x)) for tokens
            [h*4096, (h+1)*4096) of batch b (reads a 64-token halo each side).
  Host:     reshuffles y1 into per-core dilated frames.
  Launch 2: core (b,h) owns global sequences w in [32h,32h+32) of batch b
            (tokens t = w + 64p, frame ordered (w,p)); computes
            y2 = y1 + global_attn(LN2(y1)), then y3 = y2 + LN3(y2) @ ffn_w.

Matmuls in bf16 (fp32 accumulation); residual stream and softmax sums fp32.
RoPE at frame-absolute positions (scores depend only on relative position -
identical math to the reference's window-relative phases).  Softmax
denominators come from a ones-column appended to V (32-col pitch, psum
col-tiling at bases 0/32/64/96); the ones-column doubles as the key validity
mask (zero for halo padding), reproducing the reference's -1e9 key masking.
"""

import numpy as np
from contextlib import ExitStack

import concourse.bacc as bacc
import concourse.bass as bass
from concourse import mybir
from concourse.bass_utils import run_bass_kernel_spmd
from concourse.tile import TileContext

import ml_dtypes

F32 = mybir.dt.float32
BF = mybir.dt.bfloat16
AF = mybir.ActivationFunctionType
OP = mybir.AluOpType

B, T, D, H, HD = 4, 8192, 128, 8, 16
WL, WG = 128, 64
HALF = T // 2            # 4096 tokens per core
NW = 32                  # windows / sequences per core
PAD = WL // 2            # 64
FR1 = HALF + 2 * PAD     # 4224 frame rows, launch 1
NT1 = FR1 // 128         # 33 tiles
NT2 = HALF // 128        # 32 tiles
NPOS = T // WG           # 128 positions per global sequence
EPS = 1e-5
WGRP = 4                 # windows per normalize/out-proj group

_BF = ml_dtypes.bfloat16
SIM_SAFE = False   # CoreSim can't model strided-partition SBUF DMA reads


def _rope_tables(positions):
    """cos / signed-sin tables [len(positions), 16], reference _rope layout."""
    inv = 1.0 / (10000.0 ** (np.arange(0, HD, 2, dtype=np.float64) / HD))
    f = positions[:, None].astype(np.float64) * inv[None, :]
    cos = np.cos(np.concatenate([f, f], axis=1)).astype(np.float32)
    sin = np.sin(np.concatenate([f, f], axis=1)).astype(np.float32)
    ssin = sin.copy()
    ssin[:, :8] *= -1.0   # out[j<8] = x[j+8]*(-sin); out[j>=8] = x[j-8]*(+sin)
    return cos, ssin


def _ln_fold(w, wln):
    return (wln[:, None].astype(np.float64) * w.astype(np.float64)).astype(np.float32)


def _emit_ln(nc, sb, eps_sb, xt, out_bf, tag):
    """LN(x) for one [128, D] tile -> bf16 out (ln weight folded into mm weights)."""
    stats = sb.tile([128, 6], F32, tag=tag + "st")
    nc.vector.bn_stats(out=stats, in_=xt)
    mv = sb.tile([128, 2], F32, tag=tag + "mv")
    nc.vector.bn_aggr(out=mv, in_=stats)
    rstd = sb.tile([128, 1], F32, tag=tag + "rs")
    nc.scalar.activation(out=rstd, in_=mv[:, 1:2], func=AF.Sqrt,
                         bias=eps_sb, scale=1.0)
    nc.vector.reciprocal(out=rstd, in_=rstd)
    nc.vector.scalar_tensor_tensor(
        out=out_bf, in0=xt, scalar=mv[:, 0:1], in1=rstd.to_broadcast((128, D)),
        op0=OP.subtract, op1=OP.mult)


def _emit_ln_qkv_rope(nc, pools, cfg):
    """Per-tile: load x -> LN -> xlnT -> qkv matmul -> rope(q,k) -> qk_dram, v32."""
    sb, ps = pools["sb"], pools["ps_qkv"]
    ntiles = cfg["ntiles"]
    x_in, wqkv = cfg["x_in"], cfg["wqkv_sb"]
    xlnT, v32, qk_dram = cfg["xlnT"], cfg["v32"], cfg["qk_dram"]
    x_keep, eps_sb = cfg["x_keep"], cfg["eps_sb"]

    for t in range(ntiles):
        xt = sb.tile([128, D], F32, tag="xt")
        nc.sync.dma_start(out=xt, in_=x_in[t * 128:(t + 1) * 128, :])
        if x_keep is not None:
            nc.vector.tensor_copy(x_keep[:, t, :], xt)
        xln = sb.tile([128, D], BF, tag="xln")
        _emit_ln(nc, sb, eps_sb, xt, xln, "a")
        nc.scalar.dma_start(out=xlnT[:, t * 128:(t + 1) * 128], in_=xln,
                            transpose=True)
        pqkv = ps.tile([128, 3 * D], F32, tag="pqkv")
        nc.tensor.matmul(pqkv, xlnT[:, t * 128:(t + 1) * 128], wqkv,
                         start=True, stop=True)
        cos_t = cfg["cos_tile"](t)
        ssin_t = cfg["ssin_tile"](t)
        qkrot = sb.tile([128, 2, H, HD], BF, tag="qkrot")
        for j, base in enumerate((0, D)):     # j=0: q, j=1: k
            src = pqkv[:, base:base + D].rearrange("p (h d) -> p h d", h=H)
            tmp = sb.tile([128, H, HD], BF, tag="ropetmp")
            nc.vector.tensor_tensor(
                out=tmp[:, :, 0:8], in0=src[:, :, 8:16],
                in1=ssin_t[:, 0:8].unsqueeze(1).to_broadcast((128, H, 8)),
                op=OP.mult)
            nc.vector.tensor_tensor(
                out=tmp[:, :, 8:16], in0=src[:, :, 0:8],
                in1=ssin_t[:, 8:16].unsqueeze(1).to_broadcast((128, H, 8)),
                op=OP.mult)
            nc.vector.tensor_tensor(
                out=qkrot[:, j], in0=src,
                in1=cos_t.unsqueeze(1).to_broadcast((128, H, HD)), op=OP.mult)
            nc.vector.tensor_tensor(out=qkrot[:, j], in0=qkrot[:, j],
                                    in1=tmp, op=OP.add)
        nc.sync.dma_start(out=qk_dram[t * 128:(t + 1) * 128, :],
                          in_=qkrot.rearrange("p a h d -> p (a h d)"))
        nc.vector.tensor_copy(
            v32[:, t, :, 0:HD],
            pqkv[:, 2 * D:3 * D].rearrange("p (h d) -> p h d", h=H))


def _emit_attention(nc, pools, cfg):
    """scores -> exp -> av(+sums) -> normalize -> out-proj -> residual."""
    sb = pools["sb"]
    ps_score, ps_av, ps_o = pools["ps_score"], pools["ps_av"], pools["ps_o"]
    dram = pools["dram"]
    kT, qbd, v32 = cfg["kT"], cfg["qbd"], cfg["v32"]
    wo_sb = cfg["wo_sb"]
    nw, nhalf, q_off = cfg["nw"], cfg["nhalf"], cfg["q_off"]

    for wg in range(nw // WGRP):
        pav = [ps_av.tile([128, WGRP * 128], F32, tag=f"pav{i}", name=f"pav{i}") for i in range(2)]
        for wi in range(WGRP):
            w = wg * WGRP + wi
            pscore = ps_score.tile([128, nhalf, H, 128], F32, tag="pscore")
            qwin = qbd[:, w].rearrange("p h q -> p (h q)")
            psflat = pscore.rearrange("p a h q -> p (a h q)")
            for half in range(nhalf):
                k0 = cfg["key_start"](w, half)
                for c in range(2):
                    nc.tensor.matmul(
                        psflat[:, half * 1024 + c * 512:half * 1024 + (c + 1) * 512],
                        kT[:, k0:k0 + 128], qwin[:, c * 512:(c + 1) * 512],
                        start=True, stop=True)
            attn = sb.tile([128, nhalf, H, 128], BF, tag="attn")
            nc.scalar.activation(out=attn, in_=pscore, func=AF.Exp, scale=0.25)
            for h in range(H):
                pv = pav[0] if h < 4 else pav[1]
                cpos = 32 * (h % 4)
                for half in range(nhalf):
                    ktile = cfg["key_tile"](w, half)
                    nc.tensor.matmul(
                        pv[cpos:cpos + 32, wi * 128:(wi + 1) * 128],
                        v32[:, ktile, h, :], attn[:, half, h, :],
                        start=(half == 0), stop=(half == nhalf - 1),
                        tile_position=(0, cpos))
        # ---- normalize + out-proj + residual for the group ----
        araw = []
        for bi in range(2):
            ar = sb.tile([128, WGRP * 128], F32, tag=f"araw{bi}", name=f"araw{bi}")
            nc.vector.tensor_copy(ar, pav[bi])
            araw.append(ar)
        rep = []
        for bi in range(2):
            sums = sb.tile([4, WGRP * 128], F32, tag="sums")
            ar = araw[bi]
            if SIM_SAFE:
                for k in range(4):
                    nc.sync.dma_start(out=sums[k:k + 1, :],
                                      in_=ar[32 * k + 16:32 * k + 17, :])
            else:
                nc.sync.dma_start(
                    out=sums,
                    in_=ar.rearrange("(a b) n -> a b n", b=32)[:, 16, :])
            recip = sb.tile([4, WGRP * 128], F32, tag="recip")
            nc.vector.reciprocal(out=recip, in_=sums)
            rd = dram.tile([4, WGRP * 128], F32, tag="rd")
            nc.gpsimd.dma_start(out=rd[:, :], in_=recip)
            rp = sb.tile([128, WGRP * 128], F32, tag="rep")
            nc.gpsimd.dma_start(
                out=rp,
                in_=bass.AP(tensor=rd.tensor, offset=rd.offset,
                            ap=[[WGRP * 128, 4], [0, 32], [1, WGRP * 128]]))
            rep.append(rp)
        avn = []
        for bi in range(2):
            av = sb.tile([128, WGRP * 128], BF, tag="avn")
            nc.vector.tensor_tensor(out=av, in0=araw[bi], in1=rep[bi], op=OP.mult)
            avn.append(av)
        for wi in range(WGRP):
            w = wg * WGRP + wi
            po = ps_o.tile([128, D], F32, tag="po")
            nc.tensor.matmul(po, avn[0][:, wi * 128:(wi + 1) * 128], wo_sb[0],
                             start=True, stop=False)
            nc.tensor.matmul(po, avn[1][:, wi * 128:(wi + 1) * 128], wo_sb[1],
                             start=False, stop=True)
            cfg["emit_out"](w, po)


def _build_launch1(reps=1):
    nc = bacc.Bacc(None, target_bir_lowering=False)
    x_in = nc.declare_dram_parameter("xh", [FR1, D], F32, isOutput=False)
    wqkv_in = nc.declare_dram_parameter("wqkv", [D, 3 * D], BF, isOutput=False)
    wo_in = nc.declare_dram_parameter("wo_pad", [2, D, D], BF, isOutput=False)
    cos_in = nc.declare_dram_parameter("cos_t", [FR1, HD], F32, isOutput=False)
    ssin_in = nc.declare_dram_parameter("ssin_t", [FR1, HD], F32, isOutput=False)
    valid_in = nc.declare_dram_parameter("validc", [FR1, 1], F32, isOutput=False)
    y_out = nc.declare_dram_parameter("y1", [HALF, D], F32, isOutput=True)
    qk_dram = nc.dram_tensor("qk_scratch", [FR1, 2 * D], BF)

    with ExitStack() as ctx:
        tc = ctx.enter_context(TileContext(nc))
        singles = ctx.enter_context(tc.tile_pool(name="singles", bufs=1))
        sb = ctx.enter_context(tc.tile_pool(name="sb", bufs=3))
        ps_qkv = ctx.enter_context(tc.tile_pool(name="ps_qkv", bufs=1, space="PSUM"))
        ps_score = ctx.enter_context(tc.tile_pool(name="ps_score", bufs=1, space="PSUM"))
        ps_av = ctx.enter_context(tc.tile_pool(name="ps_av", bufs=1, space="PSUM"))
        ps_o = ctx.enter_context(tc.tile_pool(name="ps_o", bufs=1, space="PSUM"))
        dram = ctx.enter_context(tc.tile_pool(name="dram", bufs=2, space="DRAM"))
        pools = {"sb": sb, "ps_qkv": ps_qkv, "ps_score": ps_score,
                 "ps_av": ps_av, "ps_o": ps_o, "dram": dram}

        wqkv_sb = singles.tile([D, 3 * D], BF)
        nc.sync.dma_start(out=wqkv_sb, in_=wqkv_in[:, :])
        wo_sb = [singles.tile([D, D], BF, tag=f"wo{i}", name=f"wo{i}") for i in range(2)]
        for i in range(2):
            nc.sync.dma_start(out=wo_sb[i], in_=wo_in[i])
        cos_sb = singles.tile([128, NT1, HD], F32)
        nc.sync.dma_start(out=cos_sb, in_=cos_in.rearrange("(t p) d -> p t d", p=128))
        ssin_sb = singles.tile([128, NT1, HD], F32)
        nc.sync.dma_start(out=ssin_sb, in_=ssin_in.rearrange("(t p) d -> p t d", p=128))
        eps_sb = singles.tile([128, 1], F32)
        nc.vector.memset(eps_sb, EPS)
        xlnT = singles.tile([128, FR1], BF)
        v32 = singles.tile([128, NT1, H, 32], BF)
        qbd = singles.tile([128, NW, H, 128], BF)
        kT = singles.tile([128, FR1], BF)
        qT = singles.tile([128, FR1], BF)
        x_keep = singles.tile([128, NT1, D], F32)

        def emit_out(w, po):
            # residual: x frame rows [64+128w, 192+128w) span two x_keep tiles
            yt = sb.tile([128, D], F32, tag="yt")
            nc.vector.tensor_tensor(out=yt[0:64, :], in0=po[0:64, :],
                                    in1=x_keep[64:128, w, :], op=OP.add)
            nc.vector.tensor_tensor(out=yt[64:128, :], in0=po[64:128, :],
                                    in1=x_keep[0:64, w + 1, :], op=OP.add)
            nc.sync.dma_start(out=y_out[w * 128:(w + 1) * 128, :], in_=yt)

        def body():
            nc.gpsimd.memset(qbd, 0.0)
            nc.gpsimd.memset(v32[:, :, :, HD:32], 0.0)
            for h in range(H):
                nc.gpsimd.dma_start(
                    out=v32[:, :, h, HD:HD + 1],
                    in_=bass.AP(tensor=valid_in, offset=0,
                                ap=[[1, 128], [128, NT1], [0, 1]]))
            cfg = {
                "ntiles": NT1, "x_in": x_in, "wqkv_sb": wqkv_sb,
                "xlnT": xlnT, "v32": v32, "qk_dram": qk_dram,
                "x_keep": x_keep, "eps_sb": eps_sb,
                "cos_tile": lambda t: cos_sb[:, t, :],
                "ssin_tile": lambda t: ssin_sb[:, t, :],
            }
            _emit_ln_qkv_rope(nc, pools, cfg)
            nc.sync.dma_start(out=kT, in_=qk_dram[:, D:2 * D], transpose=True)
            nc.sync.dma_start(out=qT, in_=qk_dram[:, 0:D], transpose=True)
            for h in range(H):
                nc.gpsimd.dma_start(
                    out=qbd[16 * h:16 * (h + 1), :, h, :],
                    in_=qT[16 * h:16 * (h + 1), PAD:PAD + NW * 128].rearrange(
                        "p (w q) -> p w q", w=NW))
            acfg = {
                "kT": kT, "qbd": qbd, "v32": v32, "wo_sb": wo_sb,
                "nw": NW, "nhalf": 2, "q_off": PAD,
                "key_start": lambda w, half: w * 128 + half * 128,
                "key_tile": lambda w, half: w + half,
                "emit_out": emit_out,
            }
            _emit_attention(nc, pools, acfg)

        if reps == 1:
            body()
        else:
            with tc.For_i(0, reps, 1):
                body()
    nc.compile()
    return nc


def _build_launch2(reps=1):
    nc = bacc.Bacc(None, target_bir_lowering=False)
    x_in = nc.declare_dram_parameter("y1g", [HALF, D], F32, isOutput=False)
    wqkv_in = nc.declare_dram_parameter("wqkv", [D, 3 * D], BF, isOutput=False)
    wo_in = nc.declare_dram_parameter("wo_pad", [2, D, D], BF, isOutput=False)
    cos_in = nc.declare_dram_parameter("cos_t", [128, HD], F32, isOutput=False)
    ssin_in = nc.declare_dram_parameter("ssin_t", [128, HD], F32, isOutput=False)
    valid_in = nc.declare_dram_parameter("validc", [HALF, 1], F32, isOutput=False)
    wffn_in = nc.declare_dram_parameter("wffn", [D, D], BF, isOutput=False)
    y_out = nc.declare_dram_parameter("y3", [HALF, D], F32, isOutput=True)
    qk_dram = nc.dram_tensor("qk_scratch", [HALF, 2 * D], BF)

    with ExitStack() as ctx:
        tc = ctx.enter_context(TileContext(nc))
        singles = ctx.enter_context(tc.tile_pool(name="singles", bufs=1))
        sb = ctx.enter_context(tc.tile_pool(name="sb", bufs=3))
        ps_qkv = ctx.enter_context(tc.tile_pool(name="ps_qkv", bufs=1, space="PSUM"))
        ps_score = ctx.enter_context(tc.tile_pool(name="ps_score", bufs=2, space="PSUM"))
        ps_av = ctx.enter_context(tc.tile_pool(name="ps_av", bufs=1, space="PSUM"))
        ps_o = ctx.enter_context(tc.tile_pool(name="ps_o", bufs=1, space="PSUM"))
        dram = ctx.enter_context(tc.tile_pool(name="dram", bufs=2, space="DRAM"))
        pools = {"sb": sb, "ps_qkv": ps_qkv, "ps_score": ps_score,
                 "ps_av": ps_av, "ps_o": ps_o, "dram": dram}

        wqkv_sb = singles.tile([D, 3 * D], BF)
        nc.sync.dma_start(out=wqkv_sb, in_=wqkv_in[:, :])
        wo_sb = [singles.tile([D, D], BF, tag=f"wo{i}", name=f"wo{i}") for i in range(2)]
        for i in range(2):
            nc.sync.dma_start(out=wo_sb[i], in_=wo_in[i])
        wffn_sb = singles.tile([D, D], BF)
        nc.sync.dma_start(out=wffn_sb, in_=wffn_in[:, :])
        cos_sb = singles.tile([128, HD], F32)
        nc.sync.dma_start(out=cos_sb, in_=cos_in[:, :])
        ssin_sb = singles.tile([128, HD], F32)
        nc.sync.dma_start(out=ssin_sb, in_=ssin_in[:, :])
        eps_sb = singles.tile([128, 1], F32)
        nc.vector.memset(eps_sb, EPS)
        xlnT = singles.tile([128, HALF], BF)
        v32 = singles.tile([128, NT2, H, 32], BF)
        qbd = singles.tile([128, NW, H, 128], BF)
        kT = singles.tile([128, HALF], BF)
        qT = singles.tile([128, HALF], BF)
        x_keep = singles.tile([128, NT2, D], F32)
        y2_keep = singles.tile([128, NT2, D], F32)

        def emit_out(w, po):
            nc.vector.tensor_tensor(out=y2_keep[:, w, :], in0=po,
                                    in1=x_keep[:, w, :], op=OP.add)

        def body():
            nc.gpsimd.memset(qbd, 0.0)
            nc.gpsimd.memset(v32[:, :, :, HD:32], 0.0)
            for h in range(H):
                nc.gpsimd.dma_start(
                    out=v32[:, :, h, HD:HD + 1],
                    in_=bass.AP(tensor=valid_in, offset=0,
                                ap=[[1, 128], [128, NT2], [0, 1]]))
            cfg = {
                "ntiles": NT2, "x_in": x_in, "wqkv_sb": wqkv_sb,
                "xlnT": xlnT, "v32": v32, "qk_dram": qk_dram,
                "x_keep": x_keep, "eps_sb": eps_sb,
                "cos_tile": lambda t: cos_sb,
                "ssin_tile": lambda t: ssin_sb,
            }
            _emit_ln_qkv_rope(nc, pools, cfg)
            nc.sync.dma_start(out=kT, in_=qk_dram[:, D:2 * D], transpose=True)
            nc.sync.dma_start(out=qT, in_=qk_dram[:, 0:D], transpose=True)
            for h in range(H):
                nc.gpsimd.dma_start(
                    out=qbd[16 * h:16 * (h + 1), :, h, :],
                    in_=qT[16 * h:16 * (h + 1), :].rearrange(
                        "p (w q) -> p w q", w=NW))
            acfg = {
                "kT": kT, "qbd": qbd, "v32": v32, "wo_sb": wo_sb,
                "nw": NW, "nhalf": 1, "q_off": 0,
                "key_start": lambda w, half: w * 128,
                "key_tile": lambda w, half: w,
                "emit_out": emit_out,
            }
            _emit_attention(nc, pools, acfg)
            # ---- FFN ----
            for t in range(NT2):
                yt = y2_keep[:, t, :]
                xln3 = sb.tile([128, D], BF, tag="xln")
                _emit_ln(nc, sb, eps_sb, yt, xln3, "c")
                x3T = sb.tile([128, 128], BF, tag="x3T")
                nc.scalar.dma_start(out=x3T, in_=xln3, transpose=True)
                pf = ps_o.tile([128, D], F32, tag="po")
                nc.tensor.matmul(pf, x3T, wffn_sb, start=True, stop=True)
                yo = sb.tile([128, D], F32, tag="yo")
                nc.vector.tensor_tensor(out=yo, in0=pf, in1=yt, op=OP.add)
                nc.sync.dma_start(out=y_out[t * 128:(t + 1) * 128, :], in_=yo)

        if reps == 1:
            body()
        else:
            with tc.For_i(0, reps, 1):
                body()
    nc.compile()
    return nc


_BUILD_CACHE = {}


def _get(builder, key, reps=1):
    k = (key, reps)
    if k not in _BUILD_CACHE:
        _BUILD_CACHE[k] = builder(reps)
    return _BUILD_CACHE[k]


def _pad_wo(wo):
    """[128,128] -> [2][128,128] in the 32-pitch av layout (sum row -> 0)."""
    out = np.zeros((2, 128, 128), np.float32)
    for part in range(2):
        for k in range(4):
            h = 4 * part + k
            out[part, 32 * k:32 * k + 16, :] = wo[16 * h:16 * (h + 1), :]
    return np.ascontiguousarray(out).astype(_BF)


def _prep_host(inputs):
    p = {}
    ln1_w, ln2_w, ln3_w = inputs["ln1_w"], inputs["ln2_w"], inputs["ln3_w"]
    p["wqkv1"] = np.ascontiguousarray(np.concatenate(
        [_ln_fold(inputs["l_wq"], ln1_w), _ln_fold(inputs["l_wk"], ln1_w),
         _ln_fold(inputs["l_wv"], ln1_w)], axis=1)).astype(_BF)
    p["wo1"] = _pad_wo(np.asarray(inputs["l_wo"], np.float32))
    p["wqkv2"] = np.ascontiguousarray(np.concatenate(
        [_ln_fold(inputs["g_wq"], ln2_w), _ln_fold(inputs["g_wk"], ln2_w),
         _ln_fold(inputs["g_wv"], ln2_w)], axis=1)).astype(_BF)
    p["wo2"] = _pad_wo(np.asarray(inputs["g_wo"], np.float32))
    p["wffn"] = np.ascontiguousarray(_ln_fold(inputs["ffn_w"], ln3_w)).astype(_BF)
    return p


def _launch1_in_maps(x, m, p):
    cos1, ssin1 = _rope_tables(np.arange(FR1))
    in_maps = []
    for b in range(B):
        for h in range(2):
            lo = h * HALF - PAD
            hi = lo + FR1
            xh = np.zeros((FR1, D), np.float32)
            vc = np.zeros((FR1, 1), np.float32)
            s, e = max(lo, 0), min(hi, T)
            xh[s - lo:e - lo] = x[b, s:e]
            vc[s - lo:e - lo, 0] = m[b, s:e]
            in_maps.append({"xh": xh, "wqkv": p["wqkv1"], "wo_pad": p["wo1"],
                            "cos_t": cos1, "ssin_t": ssin1, "validc": vc})
    return in_maps


def _launch2_in_maps(y1, m, p):
    cos2, ssin2 = _rope_tables(np.arange(NPOS))
    y1r = y1.reshape(B, NPOS, WG, D)
    mr = m.reshape(B, NPOS, WG)
    in_maps = []
    for b in range(B):
        for h in range(2):
            sel = y1r[b, :, 32 * h:32 * (h + 1), :]          # [p, 32, D]
            frame = np.ascontiguousarray(
                sel.transpose(1, 0, 2).reshape(HALF, D))     # (w, p) order
            vc = np.ascontiguousarray(
                mr[b, :, 32 * h:32 * (h + 1)].T.reshape(HALF, 1)).astype(np.float32)
            in_maps.append({"y1g": frame, "wqkv": p["wqkv2"], "wo_pad": p["wo2"],
                            "cos_t": cos2, "ssin_t": ssin2, "validc": vc,
                            "wffn": p["wffn"]})
    return in_maps


def kernel(**inputs):
    x = np.asarray(inputs["x"], np.float32)
    mask = np.asarray(inputs["padding_mask"])
    m = mask.astype(np.float32)
    x = x * m[..., None]
    for nm in ("l_bq", "l_bk", "l_bv", "l_bo", "g_bq", "g_bk", "g_bv", "g_bo",
               "ffn_b", "ln1_b", "ln2_b", "ln3_b"):
        assert np.abs(np.asarray(inputs[nm])).max() == 0.0, f"nonzero bias {nm}"

    p = _prep_host(inputs)

    nc1 = _get(_build_launch1, "l1")
    res1 = run_bass_kernel_spmd(nc1, _launch1_in_maps(x, m, p),
                                core_ids=list(range(8)))
    y1 = np.empty((B, T, D), np.float32)
    for b in range(B):
        for h in range(2):
            y1[b, h * HALF:(h + 1) * HALF] = res1.results[2 * b + h]["y1"]
    y1 *= m[..., None]

    nc2 = _get(_build_launch2, "l2")
    res2 = run_bass_kernel_spmd(nc2, _launch2_in_maps(y1, m, p),
                                core_ids=list(range(8)))
    out = np.empty((B, T, D), np.float32)
    outr = out.reshape(B, NPOS, WG, D)
    for b in range(B):
        for h in range(2):
            fr = res2.results[2 * b + h]["y3"].reshape(32, NPOS, D)
            outr[b, :, 32 * h:32 * (h + 1), :] = fr.transpose(1, 0, 2)
    out *= m[..., None]
    return np.ascontiguousarray(out).astype(np.float32)
